# revision 15
# baseline (speedup 1.0000x reference)
"""GNN message-passing (2x GAT + 2x GIN, 2 edge types) on 8 trn2 NeuronCores.

v2 design — scatter-free, SWDGE-minimal:

Sharding: cores 0-3 handle edge type 0, cores 4-7 type 1. Within a quad,
nodes are sharded by dst range (12500/core, padded to 12544). Edges live on
the core owning their dst, sorted by 128-node dst block.

Per edge phase, per group of 4 dst blocks: ONE dma_gather per source
half-slice (2 total; half-slice tensors of 25088 rows keep indices int16)
fetches packed src rows token-major; a one-hot selection matrix SE[e,d] =
(dstv[e] == iota[d]) built by a single DVE compare turns the per-dst-block
aggregation into PE matmuls accumulating in PSUM (no dma_scatter_add at
all). GAT's per-edge er[dst] comes from a third gather over a replicated-row
er table (256B rows). Edge softmax needs no segment-max (logits are O(1)).

GAT0's projections are computed redundantly for ALL nodes from the
replicated feats input, so layer 0 needs no AllGather. Later AllGathers
(zel1, hcat, h3) run per row-half on half-split tensors to overlap with
compute. GIN BatchNorm stats are per-feature PSUM accumulators (ones-vector
matmuls) reduced by a tiny quad AllReduce; b1 cancels in the BN shift.
"""

import sys

for _p in ("/opt/trn_rl_repo",):
    if _p not in sys.path:
        sys.path.insert(0, _p)

import numpy as np
import ml_dtypes

import concourse.bacc as bacc
import concourse.bass as bass
import concourse.tile as tile
import concourse.mybir as mybir
from concourse.bass_utils import run_bass_kernel_spmd

FP32 = mybir.dt.float32
BF16 = mybir.dt.bfloat16
I16 = mybir.dt.int16
AF = mybir.ActivationFunctionType
ALU = mybir.AluOpType

# problem constants
N, IN, HID, H, D = 50000, 128, 256, 4, 64
E, T = 400000, 2
BN_EPS = 1e-5
P = 4                     # cores per quad
NQ = 12500                # real nodes per core
NCP = 12544               # padded (98 * 128)
HS = NCP // 2             # 6272 rows per half of a core's range
SR = P * HS               # 25088 rows per half-slice tensor
NB = NCP // 128           # 98 dst blocks
HB = NB // 2              # 49 blocks per half
GB = 4                    # dst blocks per gather group
ZW = 384                  # packed row: [z 256 | el 4 | er 4 | pad]
ERW = 128                 # replicated er row (bf16 -> 256B)
PADV = 300                # dstv pad marker (outside 0..127)
RGROUPS = [[0, 1, 2, 3], [4, 5, 6, 7]]
import os
STAGES = int(os.environ.get("GNN_STAGES", "99"))


def _bf(x):
    return np.asarray(x, dtype=ml_dtypes.bfloat16)


def _wrap_idx(a):
    """[n] ints (n % 16 == 0) -> [128, n//16] int16 SWDGE wrapped layout
    (token i at [i % 16, i // 16], replicated across the 8 Q7 cores)."""
    w = a.reshape(-1, 16).T.astype(np.int16)
    return np.tile(w, (8, 1))


def _tok_major(a):
    """[n] values (n % 128 == 0) -> [128, n//128] token-major."""
    return a.reshape(-1, 128).T


def _preprocess(inputs):
    feats = np.asarray(inputs["feats"], np.float32)
    edges = [
        (np.asarray(inputs["src0"]), np.asarray(inputs["dst0"])),
        (np.asarray(inputs["src1"]), np.asarray(inputs["dst1"])),
    ]

    # ---- edge buckets per core / dst block / src half-slice ----
    per_core = []
    for q in range(T):
        src, dst = edges[q]
        for r in range(P):
            m = (dst >= r * NQ) & (dst < (r + 1) * NQ)
            g = src[m].astype(np.int64)
            j = (dst[m] - r * NQ).astype(np.int64)
            rs = g // NQ
            is_ = g - rs * NQ
            s = is_ // HS
            row = rs * HS + (is_ - s * HS)      # row in half-slice tensor
            blk = j // 128
            buckets = {}
            for b in range(NB):
                mb_ = blk == b
                for sl in range(2):
                    sel = mb_ & (s == sl)
                    buckets[(b, sl)] = (row[sel], j[sel])
            per_core.append(buckets)

    # shared plan: per (block, slice) padded counts = max over 8 cores
    nbs = np.zeros((NB, 2), np.int64)
    for b in range(NB):
        for sl in range(2):
            mx = max(len(per_core[c][(b, sl)][0]) for c in range(8))
            nbs[b, sl] = ((mx + 127) // 128) * 128

    groups = []
    for g0 in range(0, NB, GB):
        blocks = tuple(range(g0, min(g0 + GB, NB)))
        k0 = int(sum(nbs[b, 0] for b in blocks)) // 128
        k1 = int(sum(nbs[b, 1] for b in blocks)) // 128
        slotmap = []
        for sl in range(2):
            for bi, b in enumerate(blocks):
                slotmap += [(bi, sl)] * (int(nbs[b, sl]) // 128)
        groups.append((blocks, k0, k1, tuple(slotmap)))
    plan_key = tuple(groups)

    ip_cols = []
    for (blocks, k0, k1, _) in groups:
        n0, n1 = k0 * 128, k1 * 128
        ip_cols.append(n0 // 16 + n1 // 16)
    IPW = int(np.sum(ip_cols))
    DVW = int(sum(k0 + k1 for (_, k0, k1, _) in groups))

    fpad = np.zeros((P, NCP, IN), np.float32)
    for rr in range(P):
        fpad[rr, :NQ] = feats[rr * NQ:(rr + 1) * NQ]
    feats_s = np.zeros((2, SR, IN), np.float32)
    for rr in range(P):
        feats_s[0, rr * HS:(rr + 1) * HS] = fpad[rr, 0:HS]
        feats_s[1, rr * HS:(rr + 1) * HS] = fpad[rr, HS:2 * HS]

    in_maps = []
    for c in range(8):
        q, r = c // P, c % P
        buckets = per_core[c]
        ip = np.zeros((128, IPW), np.int16)
        dv = np.zeros((128, DVW), np.float32)
        ipo = 0
        dvo = 0
        for gi, (blocks, k0, k1, _) in enumerate(groups):
            zi = [[], []]
            dvv = []
            for sl in range(2):
                for b in blocks:
                    rows, js = buckets[(b, sl)]
                    n = int(nbs[b, sl])
                    rpad = np.zeros(n, np.int64)
                    dpad = np.full(n, PADV, np.int64)
                    rpad[: len(rows)] = rows
                    dpad[: len(js)] = js - b * 128
                    zi[sl].append(rpad)
                    dvv.append(dpad)
            z0 = (np.concatenate(zi[0]) if zi[0] else np.zeros(0, np.int64))
            z1 = (np.concatenate(zi[1]) if zi[1] else np.zeros(0, np.int64))
            dvs = np.concatenate(dvv)
            for arr in (z0, z1):
                if len(arr):
                    w = _wrap_idx(arr)
                    ip[:, ipo:ipo + w.shape[1]] = w
                    ipo += w.shape[1]
            ns = len(dvs) // 128
            dv[:, dvo:dvo + ns] = _tok_major(dvs.astype(np.float32))
            dvo += ns
        assert ipo == IPW and dvo == DVW, (ipo, IPW, dvo, DVW)

        def gat_wx(Wt, al, ar):
            Wr = Wt.reshape(Wt.shape[0], H, D)
            wal = np.einsum("khd,hd->kh", Wr, al)
            war = np.einsum("khd,hd->kh", Wr, ar)
            wx = np.concatenate([Wt, wal, war], 1)          # [F_in, 264]
            kc = wx.shape[0] // 128
            return _bf(np.ascontiguousarray(
                wx.reshape(kc, 128, 264).transpose(1, 0, 2)))

        def wchunks(Wt):
            kc = Wt.shape[0] // 128
            return _bf(np.ascontiguousarray(
                Wt.reshape(kc, 128, Wt.shape[1]).transpose(1, 0, 2)))

        def fvec(v):
            return np.ascontiguousarray(
                np.asarray(v, np.float32).reshape(2, 128)
                .transpose(1, 0)[:, :, None])

        g = lambda k: np.asarray(inputs[k], np.float32)

        ers0 = _wrap_idx(np.arange(r * HS, (r + 1) * HS, dtype=np.int64))
        ers1 = ers0.copy()

        m = {
            "feats_a0": _bf(feats_s[0]),
            "feats_a1": _bf(feats_s[1]),
            "feats_loc": _bf(fpad[r]),
            "idxpack": ip,
            "dstv": dv.astype(ml_dtypes.bfloat16),
            "ersrc0": ers0,
            "ersrc1": ers1,
            "w0x": gat_wx(g("gat0_W")[q], g("gat0_al")[q], g("gat0_ar")[q]),
            "w1x": gat_wx(g("gat1_W")[q], g("gat1_al")[q], g("gat1_ar")[q]),
            "b0": np.tile(g("gat0_b")[q][None, :], (128, 1)).astype(np.float32),
            "b1": np.tile(g("gat1_b")[q][None, :], (128, 1)).astype(np.float32),
            "g0w1": wchunks(g("gin0_W1")[q]),
            "g0w2": wchunks(g("gin0_W2")[q]),
            "g1w1": wchunks(g("gin1_W1")[q]),
            "g1w2": wchunks(g("gin1_W2")[q]),
            "g0g1": fvec(g("gin0_g1")[q]),
            "g0be1": fvec(g("gin0_be1")[q]),
            "g1g1": fvec(g("gin1_g1")[q]),
            "g1be1": fvec(g("gin1_be1")[q]),
            "g0b2t": np.tile(g("gin0_b2")[q][None, :], (128, 1)).astype(np.float32),
            "g1b2t": np.tile(g("gin1_b2")[q][None, :], (128, 1)).astype(np.float32),
            "eps0": np.full((128, 1), 1.0 + float(g("gin0_eps")[q]), np.float32),
            "eps1": np.full((128, 1), 1.0 + float(g("gin1_eps")[q]), np.float32),
            "identity": _bf(np.eye(128)),
            "identity_f": np.eye(128, dtype=np.float32),
            "iota": _bf(np.tile(np.arange(128, dtype=np.float32)[None, :],
                                (128, 1))),
            "ones_col": _bf(np.ones((128, 1), np.float32)),
            "ones_row": np.ones((1, 128), np.float32),
            "padmask": np.concatenate([
                np.ones((NQ - (NB - 1) * 128, 1), np.float32),
                np.zeros((NCP - NQ, 1), np.float32)]),
        }
        in_maps.append(m)
    return in_maps, (plan_key, IPW, DVW)


def _rows(dram, r0, nt, width):
    return dram[r0 * 128:(r0 + nt) * 128, :].rearrange("(t p) f -> p t f", p=128)


def build_program(plan):
    plan_key, IPW, DVW = plan
    groups = list(plan_key)   # (blocks, k0, k1, slotmap)

    nc = bacc.Bacc("TRN2", target_bir_lowering=False, debug=False,
                   num_devices=8, dynamic_dma_scratch_size=32768)

    dp = nc.declare_dram_parameter
    feats_a = [dp("feats_a0", [SR, IN], BF16, isOutput=False),
               dp("feats_a1", [SR, IN], BF16, isOutput=False)]
    feats_loc_d = dp("feats_loc", [NCP, IN], BF16, isOutput=False)
    ip_d = dp("idxpack", [128, IPW], I16, isOutput=False)
    dv_d = dp("dstv", [128, DVW], BF16, isOutput=False)
    ersrc_d = [dp("ersrc0", [128, HS // 16], I16, isOutput=False),
               dp("ersrc1", [128, HS // 16], I16, isOutput=False)]
    w0x_d = dp("w0x", [128, 1, 264], BF16, isOutput=False)
    w1x_d = dp("w1x", [128, 2, 264], BF16, isOutput=False)
    b0_d = dp("b0", [128, HID], FP32, isOutput=False)
    b1_d = dp("b1", [128, HID], FP32, isOutput=False)
    g0w1_d = dp("g0w1", [128, 3, HID], BF16, isOutput=False)
    g0w2_d = dp("g0w2", [128, 2, HID], BF16, isOutput=False)
    g1w1_d = dp("g1w1", [128, 2, HID], BF16, isOutput=False)
    g1w2_d = dp("g1w2", [128, 2, HID], BF16, isOutput=False)
    vec_d = {nm: dp(nm, [128, 2, 1], FP32, isOutput=False)
             for nm in ("g0g1", "g0be1", "g1g1", "g1be1")}
    b2t_d = {nm: dp(nm, [128, HID], FP32, isOutput=False)
             for nm in ("g0b2t", "g1b2t")}
    eps0_d = dp("eps0", [128, 1], FP32, isOutput=False)
    eps1_d = dp("eps1", [128, 1], FP32, isOutput=False)
    ident_d = dp("identity", [128, 128], BF16, isOutput=False)
    identf_d = dp("identity_f", [128, 128], FP32, isOutput=False)
    iota_d = dp("iota", [128, 128], BF16, isOutput=False)
    onesc_d = dp("ones_col", [128, 1], BF16, isOutput=False)
    onesr_d = dp("ones_row", [1, 128], FP32, isOutput=False)
    padmask_d = dp("padmask", [128, 1], FP32, isOutput=False)

    out_d = dp("out", [NCP, HID], FP32, isOutput=True)

    # DRAM scratch. *_loc tensors are split in row halves so each
    # AllGather half only depends on the blocks that feed it.
    zel0_s = [nc.dram_tensor(f"zel0_s{i}", [SR, ZW], BF16) for i in range(2)]
    zel1_s = [nc.dram_tensor(f"zel1_s{i}", [SR, ZW], BF16) for i in range(2)]
    hcat_s = [nc.dram_tensor(f"hcat_s{i}", [SR, ZW], BF16) for i in range(2)]
    h3_s = [nc.dram_tensor(f"h3_s{i}", [SR, HID], BF16) for i in range(2)]
    zel1_loc = [nc.dram_tensor(f"zel1_loc{i}", [HS, ZW], BF16)
                for i in range(2)]
    hcat_loc = [nc.dram_tensor(f"hcat_loc{i}", [HS, ZW], BF16)
                for i in range(2)]
    h3_loc = [nc.dram_tensor(f"h3_loc{i}", [HS, HID], BF16)
              for i in range(2)]
    er_cmp = [nc.dram_tensor(f"er_cmp{i}", [NCP, 4], BF16)
              for i in range(2)]
    arb_in = [nc.dram_tensor(f"arb_in{i}", [128, 4], FP32) for i in range(2)]
    scl_dram = [nc.dram_tensor(f"scl_dram{i}", [4, 128], FP32)
                for i in range(2)]
    arb_out = [nc.dram_tensor(f"arb_out{i}", [128, 4], FP32) for i in range(2)]

    def loc_rows(halves, b, width):
        """[128, width] AP for dst-block b of a half-split row tensor."""
        half, bb = (0, b) if b < HB else (1, b - HB)
        return halves[half][bb * 128:(bb + 1) * 128, 0:width].rearrange(
            "(t p) f -> p t f", p=128)[:, 0, :]

    ip_off, dv_off = [], []
    o1, o2 = 0, 0
    for (blocks, k0, k1, _) in groups:
        ip_off.append(o1)
        dv_off.append(o2)
        n0, n1 = k0 * 128, k1 * 128
        o1 += n0 // 16 + n1 // 16
        o2 += k0 + k1
    maxslots = max(k0 + k1 for (_, k0, k1, _) in groups)

    with tile.TileContext(nc) as tc:
        cst = tc.alloc_tile_pool(name="cst", bufs=1)

        def ld(dram, shape, dtype):
            t = cst.tile(shape, dtype, tag=dram.name + "_sb")
            nc.sync.dma_start(out=t[:],
                              in_=dram[tuple(slice(None) for _ in shape)])
            return t

        ident = ld(ident_d, [128, 128], BF16)
        identf = ld(identf_d, [128, 128], FP32)
        iota = ld(iota_d, [128, 128], BF16)
        onesc = ld(onesc_d, [128, 1], BF16)
        onesr = ld(onesr_d, [1, 128], FP32)
        padmask = ld(padmask_d, [128, 1], FP32)
        w0x = ld(w0x_d, [128, 1, 264], BF16)
        w1x = ld(w1x_d, [128, 2, 264], BF16)
        b0 = ld(b0_d, [128, HID], FP32)
        b1 = ld(b1_d, [128, HID], FP32)
        g0w1 = ld(g0w1_d, [128, 3, HID], BF16)
        g0w2 = ld(g0w2_d, [128, 2, HID], BF16)
        g1w1 = ld(g1w1_d, [128, 2, HID], BF16)
        g1w2 = ld(g1w2_d, [128, 2, HID], BF16)
        vec = {nm: ld(d, [128, 2, 1], FP32) for nm, d in vec_d.items()}
        b2t = {nm: ld(d, [128, HID], FP32) for nm, d in b2t_d.items()}
        eps0 = ld(eps0_d, [128, 1], FP32)
        eps1 = ld(eps1_d, [128, 1], FP32)

        big = tc.alloc_tile_pool(name="big", bufs=1)
        x1_sb = big.tile([128, NB, HID], BF16, tag="x1_sb")

        # ---------------- GAT0 node: all nodes, no AG ----------------
        def gat0_node():
            with tc.tile_pool(name="n0", bufs=3) as pool, \
                 tc.tile_pool(name="n0p", bufs=2, space="PSUM") as pp:
                for sl in range(2):
                    ntile = SR // 128        # 196
                    for t0 in range(0, ntile, 4):
                        nt = min(4, ntile - t0)
                        ft = pool.tile([128, 4, IN], BF16, tag="ft")
                        nc.sync.dma_start(out=ft[:, 0:nt, :],
                                          in_=_rows(feats_a[sl], t0, nt, IN))
                        zel = pool.tile([128, 4, 264], BF16, tag="zel")
                        for t in range(nt):
                            pt = pp.tile([128, 128], BF16, tag="tp")
                            nc.tensor.transpose(out=pt[:], in_=ft[:, t, :],
                                                identity=ident[:])
                            fT = pool.tile([128, 128], BF16, tag="fT")
                            nc.any.tensor_copy(out=fT[:], in_=pt[:])
                            zp = pp.tile([128, 512], FP32, tag="zp")
                            nc.tensor.matmul(zp[:, 0:264], lhsT=fT[:],
                                             rhs=w0x[:, 0, :],
                                             start=True, stop=True)
                            nc.any.tensor_copy(out=zel[:, t, :],
                                               in_=zp[:, 0:264])
                        nc.sync.dma_start(
                            out=zel0_s[sl][t0 * 128:(t0 + nt) * 128, 0:264]
                            .rearrange("(t p) f -> p t f", p=128),
                            in_=zel[:, 0:nt, :])

        # er_cmp0[i] <- zel0_s[sl][own rows, 260:264]
        def er_fill():
            with tc.tile_pool(name="ef", bufs=2) as pool:
                for sl in range(2):
                    st = pool.tile([128, HS // 16], I16, tag="efst")
                    nc.sync.dma_start(out=st[:], in_=ersrc_d[sl][:, :])
                    zg = pool.tile([128, HS // 128, ZW], BF16, tag="efzg")
                    for p0 in range(0, HS, 896):
                        nc.gpsimd.dma_gather(
                            zg[:, p0 // 128:(p0 + 896) // 128, :],
                            zel0_s[sl][:, :],
                            st[:, p0 // 16:(p0 + 896) // 16], 896, 896, ZW)
                    erb = pool.tile([128, HS // 128, 4], BF16, tag="efb")
                    nc.vector.tensor_copy(out=erb[:], in_=zg[:, :, 260:264])
                    nc.sync.dma_start(
                        out=er_cmp[0][sl * HS:(sl + 1) * HS, :].rearrange(
                            "(t p) f -> p t f", p=128),
                        in_=erb[:])

        # hcat_loc cols 256:384 <- feats_loc
        def hcat_prefill():
            with tc.tile_pool(name="hp", bufs=2) as pool:
                for half in range(2):
                    for t0 in range(0, HB, 7):
                        ftl = pool.tile([128, 7, IN], BF16, tag="ftl")
                        nc.sync.dma_start(
                            out=ftl[:],
                            in_=_rows(feats_loc_d, half * HB + t0, 7, IN))
                        nc.sync.dma_start(
                            out=hcat_loc[half][t0 * 128:(t0 + 7) * 128,
                                               256:384]
                            .rearrange("(t p) f -> p t f", p=128),
                            in_=ftl[:])

        MAXTOK = 1024   # SWDGE ring holds 1024 descriptors

        def gather_split(zg, src_ap, ipt, col0, slot0, ntok, width):
            """dma_gather of ntok tokens in <=MAXTOK pieces (slot-aligned)."""
            done = 0
            while done < ntok:
                take = min(MAXTOK, ntok - done)
                s0 = slot0 + done // 128
                s1 = s0 + (take + 127) // 128
                nc.gpsimd.dma_gather(
                    zg[:, s0:s1, :], src_ap,
                    ipt[:, col0 + done // 16:col0 + (done + take) // 16],
                    take, take, width)
                done += take

        # ---------------- edge phase ----------------
        def edge_phase(layer, src_s, width, er_src, post, mid_cb=None):
            gat = layer < 2
            rw = 264 if gat else width
            with tc.tile_pool(name=f"e{layer}", bufs=2) as pool, \
                 tc.tile_pool(name=f"e{layer}q", bufs=2) as poolq, \
                 tc.tile_pool(name=f"e{layer}r", bufs=1, space="PSUM") as ppr, \
                 tc.tile_pool(name=f"e{layer}x", bufs=1, space="PSUM") as ppx, \
                 tc.tile_pool(name=f"e{layer}p", bufs=2, space="PSUM") as pp:
                for gi, (blocks, k0, k1, slotmap) in enumerate(groups):
                    ks = k0 + k1
                    n0, n1 = k0 * 128, k1 * 128
                    ipw = n0 // 16 + n1 // 16
                    ipt = poolq.tile([128, ipw], I16, tag="ipt")
                    nc.sync.dma_start(
                        out=ipt[:], in_=ip_d[:, ip_off[gi]:ip_off[gi] + ipw])
                    dvt = poolq.tile([128, maxslots], BF16, tag="dvt")
                    nc.sync.dma_start(
                        out=dvt[:, 0:ks],
                        in_=dv_d[:, dv_off[gi]:dv_off[gi] + ks])
                    zg = pool.tile([128, maxslots, width], BF16, tag="zg")
                    if k0:
                        gather_split(zg, src_s[0][:, :], ipt, 0, 0, n0, width)
                    if k1:
                        gather_split(zg, src_s[1][:, :], ipt, n0 // 16, k0,
                                     n1, width)
                    se = pool.tile([128, maxslots, 128], BF16, tag="se")
                    nc.vector.tensor_tensor(
                        out=se[:, 0:ks, :],
                        in0=dvt[:, 0:ks].unsqueeze(2).broadcast_to(
                            [128, ks, 128]),
                        in1=iota[:].unsqueeze(1).broadcast_to([128, ks, 128]),
                        op=ALU.is_equal)
                    if gat:
                        # er[dst] per token: one-hot SE_T x er_blk on PE
                        erb = poolq.tile([128, GB, 4], BF16, tag="erb")
                        nblk_ = len(blocks)
                        nc.sync.dma_start(
                            out=erb[:, 0:nblk_, :],
                            in_=er_src[blocks[0] * 128:
                                       (blocks[0] + nblk_) * 128, :]
                            .rearrange("(t p) f -> p t f", p=128))
                        seT = pool.tile([128, maxslots, 128], BF16, tag="seT")
                        erp = ppr.tile([128, 512], FP32, tag="erp")
                        for slot, (bi, sl) in enumerate(slotmap):
                            ptT = pp.tile([128, 128], BF16, tag="tp1")
                            nc.tensor.transpose(out=ptT[:],
                                                in_=se[:, slot, :],
                                                identity=ident[:])
                            nc.scalar.copy(out=seT[:, slot, :], in_=ptT[:])
                            nc.tensor.matmul(
                                erp[:, 4 * slot:4 * slot + 4],
                                lhsT=seT[:, slot, :], rhs=erb[:, bi, :],
                                start=(slot == 0), stop=(slot == ks - 1))
                        lg = pool.tile([128, maxslots, H], FP32, tag="lg")
                        nc.vector.tensor_tensor(
                            out=lg[:, 0:ks, :], in0=zg[:, 0:ks, 256:260],
                            in1=erp[:, 0:4 * ks].rearrange(
                                "p (s f) -> p s f", f=4),
                            op=ALU.add)
                        lr = pool.tile([128, maxslots, H], FP32, tag="lr")
                        nc.vector.scalar_tensor_tensor(
                            out=lr[:, 0:ks, :], in0=lg[:, 0:ks, :],
                            scalar=0.2, in1=lg[:, 0:ks, :],
                            op0=ALU.mult, op1=ALU.max)
                        wt = pool.tile([128, maxslots, H], BF16, tag="wt")
                        nc.scalar.activation(out=wt[:, 0:ks, :],
                                             in_=lr[:, 0:ks, :], func=AF.Exp)
                        nc.vector.tensor_tensor(
                            out=zg[:, 0:ks, 0:256].rearrange(
                                "p s (h d) -> p s h d", h=H),
                            in0=zg[:, 0:ks, 0:256].rearrange(
                                "p s (h d) -> p s h d", h=H),
                            in1=wt[:, 0:ks, :].unsqueeze(3).broadcast_to(
                                [128, ks, H, D]),
                            op=ALU.mult)
                        nc.vector.tensor_copy(out=zg[:, 0:ks, 256:260],
                                              in_=wt[:, 0:ks, :])
                    nblk = len(blocks)
                    pbs = [ppr.tile([128, 512], FP32, tag=f"rst{bi}",
                                    name=f"rst{bi}")
                           for bi in range(nblk)]
                    first = [True] * nblk
                    last_slot = {}
                    for slot, (bi, sl) in enumerate(slotmap):
                        last_slot[bi] = slot
                    for slot, (bi, sl) in enumerate(slotmap):
                        nc.tensor.matmul(
                            pbs[bi][:, 0:rw],
                            lhsT=se[:, slot, :], rhs=zg[:, slot, 0:rw],
                            start=first[bi], stop=(slot == last_slot[bi]))
                        first[bi] = False
                    for bi, b in enumerate(blocks):
                        post(b, pbs[bi], pool, pp, ppx)
                    if mid_cb is not None and gi in mid_cb:
                        mid_cb[gi]()

        # ---------------- posts ----------------
        def gat_post(layer):
            bias = b0 if layer == 0 else b1

            def post(b, pb, pool, pp, ppx):
                dmax = pool.tile([128, H], FP32, tag="dmax")
                nc.vector.tensor_scalar_max(dmax[:], pb[:, 256:260], 1e-9)
                rec = pool.tile([128, H], FP32, tag="rec")
                nc.vector.reciprocal(rec[:], dmax[:])
                hb = pool.tile([128, HID], FP32, tag="hb")
                nc.vector.tensor_tensor(
                    out=hb[:].rearrange("p (h d) -> p h d", h=H),
                    in0=pb[:, 0:256].rearrange("p (h d) -> p h d", h=H),
                    in1=rec[:].unsqueeze(2).broadcast_to([128, H, D]),
                    op=ALU.mult)
                hb2 = pool.tile([128, HID], FP32, tag="hb2")
                nc.vector.tensor_tensor(out=hb2[:], in0=hb[:], in1=bias[:],
                                        op=ALU.add)
                hf = pool.tile([128, HID], BF16, tag="hf")
                nc.scalar.activation(out=hf[:], in_=hb2[:], func=AF.Relu)
                if layer == 0:
                    # fused GAT1 node: zel1 = h1 @ w1x
                    hT = pool.tile([128, 2, 128], BF16, tag="hT")
                    for k2 in range(2):
                        pt = pp.tile([128, 128], BF16, tag="tp1")
                        nc.tensor.transpose(
                            out=pt[:], in_=hf[:, k2 * 128:(k2 + 1) * 128],
                            identity=ident[:])
                        nc.any.tensor_copy(out=hT[:, k2, :], in_=pt[:])
                    zp = ppx.tile([128, 512], FP32, tag="z1p")
                    for k2 in range(2):
                        nc.tensor.matmul(zp[:, 0:264], lhsT=hT[:, k2, :],
                                         rhs=w1x[:, k2, :],
                                         start=(k2 == 0), stop=(k2 == 1))
                    z1f = pool.tile([128, 264], BF16, tag="z1f")
                    nc.any.tensor_copy(out=z1f[:], in_=zp[:, 0:264])
                    nc.sync.dma_start(out=loc_rows(zel1_loc, b, 264),
                                      in_=z1f[:])
                    nc.sync.dma_start(
                        out=er_cmp[1][b * 128:(b + 1) * 128, :].rearrange(
                            "(t p) f -> p t f", p=128)[:, 0, :],
                        in_=z1f[:, 260:264])
                else:
                    nc.sync.dma_start(out=loc_rows(hcat_loc, b, 256),
                                      in_=hf[:])
            return post

        def gin_post(layer, stats_pb):
            gidx = layer - 2
            w1 = g0w1 if gidx == 0 else g1w1
            epsv = eps0 if gidx == 0 else eps1
            hc_src = hcat_loc if gidx == 0 else h3_loc
            w_in = 384 if gidx == 0 else 256
            kc = w_in // 128

            def post(b, pb, pool, pp, ppx):
                hcin = pool.tile([128, w_in], BF16, tag="hcin")
                nc.sync.dma_start(out=hcin[:], in_=loc_rows(hc_src, b, w_in))
                xc = pool.tile([128, w_in], BF16, tag="xc")
                nc.vector.scalar_tensor_tensor(
                    out=xc[:], in0=hcin[:], scalar=epsv[:],
                    in1=pb[:, 0:w_in], op0=ALU.mult, op1=ALU.add)
                if b == NB - 1:
                    # zero pad nodes 12500..12543 (partitions 84..127)
                    nc.vector.tensor_tensor(
                        out=xc[:], in0=xc[:],
                        in1=padmask[:].broadcast_to([128, w_in]),
                        op=ALU.mult)
                xT = pool.tile([128, 3, 128], BF16, tag="xT")
                for k2 in range(kc):
                    pt = pp.tile([128, 128], BF16, tag="tp2")
                    nc.tensor.transpose(
                        out=pt[:], in_=xc[:, k2 * 128:(k2 + 1) * 128],
                        identity=ident[:])
                    nc.any.tensor_copy(out=xT[:, k2, :], in_=pt[:])
                xp = ppx.tile([128, 512], FP32, tag="x1p")
                for k2 in range(kc):
                    nc.tensor.matmul(xp[:, 0:HID], lhsT=xT[:, k2, :],
                                     rhs=w1[:, k2, :],
                                     start=(k2 == 0), stop=(k2 == kc - 1))
                x1f = pool.tile([128, HID], BF16, tag="x1f")
                nc.any.tensor_copy(out=x1f[:], in_=xp[:, 0:HID])
                nc.vector.tensor_copy(out=x1_sb[:, b, :], in_=x1f[:])
                sq = pool.tile([128, HID], BF16, tag="sq")
                nc.scalar.activation(out=sq[:], in_=xp[:, 0:HID],
                                     func=AF.Square)
                for col, (srct, chk) in enumerate(
                        ((x1f, 0), (x1f, 1), (sq, 0), (sq, 1))):
                    nc.tensor.matmul(
                        stats_pb[:, col:col + 1],
                        lhsT=srct[:, chk * 128:(chk + 1) * 128], rhs=onesc[:],
                        start=(b == 0 and col == 0),
                        stop=(b == NB - 1 and col == 3))
            return post

        def gin_finish(layer):
            gidx = layer - 2
            w2 = g0w2 if gidx == 0 else g1w2
            pre = "g0" if gidx == 0 else "g1"
            out_f32 = gidx == 1
            with tc.tile_pool(name=f"f{layer}", bufs=3) as pool, \
                 tc.tile_pool(name=f"f{layer}p", bufs=2, space="PSUM") as pp:
                art = pool.tile([128, 4], FP32, tag="art")
                nc.sync.dma_start(out=art[:], in_=arb_out[gidx][:, :])
                mu = pool.tile([128, 2], FP32, tag="mu")
                nc.vector.tensor_scalar_mul(mu[:], art[:, 0:2], 1.0 / N)
                msq = pool.tile([128, 2], FP32, tag="msq")
                nc.vector.tensor_scalar_mul(msq[:], art[:, 2:4], 1.0 / N)
                mu2 = pool.tile([128, 2], FP32, tag="mu2")
                nc.vector.tensor_mul(mu2[:], mu[:], mu[:])
                var = pool.tile([128, 2], FP32, tag="var")
                nc.vector.tensor_sub(var[:], msq[:], mu2[:])
                vare = pool.tile([128, 2], FP32, tag="vare")
                nc.vector.tensor_scalar_add(vare[:], var[:], BN_EPS)
                sd = pool.tile([128, 2], FP32, tag="sd")
                nc.scalar.activation(out=sd[:], in_=vare[:], func=AF.Sqrt)
                rsd = pool.tile([128, 2], FP32, tag="rsd")
                nc.vector.reciprocal(rsd[:], sd[:])
                scl4 = pool.tile([128, 4], FP32, tag="scl4")
                nc.vector.tensor_mul(scl4[:, 0:2], rsd[:],
                                     vec[pre + "g1"][:, :, 0])
                mus = pool.tile([128, 2], FP32, tag="mus")
                nc.vector.tensor_mul(mus[:], mu[:], scl4[:, 0:2])
                nc.vector.tensor_sub(scl4[:, 2:4], vec[pre + "be1"][:, :, 0],
                                     mus[:])
                # broadcast feature-major [128, 4] -> token-major [128, 256]
                ptT = pp.tile([4, 128], FP32, tag="sclTp")
                nc.tensor.transpose(out=ptT[:], in_=scl4[:], identity=identf[:])
                scr = pool.tile([4, 128], FP32, tag="scr")
                nc.any.tensor_copy(out=scr[:], in_=ptT[:])
                # roundtrip rows through DRAM to land each at partition 0
                nc.sync.dma_start(out=scl_dram[gidx][:, :], in_=scr[:])
                sclT = pool.tile([128, HID], FP32, tag="ssclT")
                shfT = pool.tile([128, HID], FP32, tag="sshfT")
                for row, dstt in ((0, sclT), (1, sclT), (2, shfT), (3, shfT)):
                    chk = row % 2
                    srow = pool.tile([1, 128], FP32, tag=f"srow{row}",
                                     name=f"srow{row}")
                    nc.sync.dma_start(out=srow[:],
                                      in_=scl_dram[gidx][row:row + 1, :])
                    bp = pp.tile([128, 128], FP32, tag="bp")
                    nc.tensor.matmul(bp[:], lhsT=onesr[:, :], rhs=srow[:],
                                     start=True, stop=True)
                    nc.any.tensor_copy(out=dstt[:, chk * 128:(chk + 1) * 128],
                                       in_=bp[:])
                # pass B over x1_sb; 7-block tiles stay within row halves
                passb_tiles = list(range(0, NB, 7))
                for t0 in passb_tiles:
                    x1n = pool.tile([128, 7, HID], BF16, tag="x1n")
                    nc.vector.tensor_tensor(
                        out=x1n[:], in0=x1_sb[:, t0:t0 + 7, :],
                        in1=sclT[:].unsqueeze(1).broadcast_to([128, 7, HID]),
                        op=ALU.mult)
                    nc.vector.tensor_tensor(
                        out=x1n[:], in0=x1n[:],
                        in1=shfT[:].unsqueeze(1).broadcast_to([128, 7, HID]),
                        op=ALU.add)
                    nc.scalar.activation(out=x1n[:], in_=x1n[:], func=AF.Relu)
                    ho = pool.tile([128, 7, HID], FP32 if out_f32 else BF16,
                                   tag="ho")
                    for t in range(7):
                        xT = pool.tile([128, 2, 128], BF16, tag="xT2")
                        for k2 in range(2):
                            pt2 = pp.tile([128, 128], BF16, tag="tp3")
                            nc.tensor.transpose(
                                out=pt2[:],
                                in_=x1n[:, t, k2 * 128:(k2 + 1) * 128],
                                identity=ident[:])
                            nc.any.tensor_copy(out=xT[:, k2, :], in_=pt2[:])
                        x2p = pp.tile([128, 512], FP32, tag="x2p")
                        for k2 in range(2):
                            nc.tensor.matmul(x2p[:, 0:HID], lhsT=xT[:, k2, :],
                                             rhs=w2[:, k2, :],
                                             start=(k2 == 0), stop=(k2 == 1))
                        hb3 = pool.tile([128, HID], FP32, tag="hb3")
                        nc.vector.tensor_tensor(out=hb3[:], in0=x2p[:, 0:HID],
                                                in1=b2t[pre + "b2t"][:],
                                                op=ALU.add)
                        nc.scalar.activation(out=ho[:, t, :], in_=hb3[:],
                                             func=AF.Relu)
                    if out_f32:
                        nc.sync.dma_start(out=_rows(out_d, t0, 7, HID),
                                          in_=ho[:])
                    else:
                        half, tt = (0, t0) if t0 < HB else (1, t0 - HB)
                        nc.sync.dma_start(
                            out=_rows(h3_loc[half], tt, 7, HID), in_=ho[:])
                        if t0 + 7 == HB:
                            allgather_half(h3_loc, h3_s, 0)
                        elif t0 + 7 == NB:
                            allgather_half(h3_loc, h3_s, 1)

        def allgather_half(src_halves, dsts, half):
            nc.gpsimd.collective_compute(
                "AllGather", ALU.bypass, replica_groups=RGROUPS,
                ins=[src_halves[half][:, :].opt()],
                outs=[dsts[half][:, :].opt()])

        def allreduce_stats(gidx, stats_pb):
            with tc.tile_pool(name=f"ar{gidx}", bufs=1) as pool:
                arp = pool.tile([128, 4], FP32, tag="arp")
                nc.vector.tensor_copy(out=arp[:], in_=stats_pb[:, 0:4])
                nc.sync.dma_start(out=arb_in[gidx][:, :], in_=arp[:])
            nc.gpsimd.collective_compute(
                "AllReduce", ALU.add, replica_groups=RGROUPS,
                ins=[arb_in[gidx][:, :].opt()],
                outs=[arb_out[gidx][:, :].opt()])

        # ---------------- schedule ----------------
        # half 0 = dst blocks 0..48; group 12 = blocks 48..51 completes it
        gat0_node()
        hcat_prefill()
        er_fill()
        if STAGES >= 2:
            edge_phase(0, zel0_s, ZW, er_cmp[0], gat_post(0),
                       mid_cb={12: lambda: allgather_half(zel1_loc, zel1_s, 0),
                               24: lambda: allgather_half(zel1_loc, zel1_s, 1)})
        if STAGES >= 3:
            edge_phase(1, zel1_s, ZW, er_cmp[1], gat_post(1),
                       mid_cb={12: lambda: allgather_half(hcat_loc, hcat_s, 0),
                               24: lambda: allgather_half(hcat_loc, hcat_s, 1)})
        if STAGES >= 4:
            with tc.tile_pool(name="sp0", bufs=1, space="PSUM") as sp:
                stats0 = sp.tile([128, 512], FP32, tag="stats0")
                edge_phase(2, hcat_s, ZW, None, gin_post(2, stats0))
                allreduce_stats(0, stats0)
            gin_finish(2)
        if STAGES >= 5:
            with tc.tile_pool(name="sp1", bufs=1, space="PSUM") as sp:
                stats1 = sp.tile([128, 512], FP32, tag="stats1")
                edge_phase(3, h3_s, HID, None, gin_post(3, stats1))
                allreduce_stats(1, stats1)
            gin_finish(3)

        big.release()
        cst.release()

    nc.compile()
    return nc


_CACHE = {}


def kernel(**inputs):
    in_maps, plan = _preprocess(inputs)
    nc = _CACHE.get(plan[0])
    if nc is None:
        nc = build_program(plan)
        _CACHE[plan[0]] = nc
    res = run_bass_kernel_spmd(nc, in_maps, core_ids=list(range(8)))
    out = np.zeros((N, T * HID), np.float32)
    for c in range(8):
        q, r = c // P, c % P
        out[r * NQ:(r + 1) * NQ, q * HID:(q + 1) * HID] = \
            np.asarray(res.results[c]["out"], np.float32)[:NQ]
    return out


# revision 16
# speedup vs baseline: 1.1852x; 1.1852x over previous
"""GNN message-passing (2x GAT + 2x GIN, 2 edge types) on 8 trn2 NeuronCores.

v2 design — scatter-free, SWDGE-minimal:

Sharding: cores 0-3 handle edge type 0, cores 4-7 type 1. Within a quad,
nodes are sharded by dst range (12500/core, padded to 12544). Edges live on
the core owning their dst, sorted by 128-node dst block.

Per edge phase, per group of 4 dst blocks: ONE dma_gather per source
half-slice (2 total; half-slice tensors of 25088 rows keep indices int16)
fetches packed src rows token-major; a one-hot selection matrix SE[e,d] =
(dstv[e] == iota[d]) built by a single DVE compare turns the per-dst-block
aggregation into PE matmuls accumulating in PSUM (no dma_scatter_add at
all). GAT's per-edge er[dst] comes from a third gather over a replicated-row
er table (256B rows). Edge softmax needs no segment-max (logits are O(1)).

GAT0's projections are computed redundantly for ALL nodes from the
replicated feats input, so layer 0 needs no AllGather. Later AllGathers
(zel1, hcat, h3) run per row-half on half-split tensors to overlap with
compute. GIN BatchNorm stats are per-feature PSUM accumulators (ones-vector
matmuls) reduced by a tiny quad AllReduce; b1 cancels in the BN shift.
"""

import sys

for _p in ("/opt/trn_rl_repo",):
    if _p not in sys.path:
        sys.path.insert(0, _p)

import numpy as np
import ml_dtypes

import concourse.bacc as bacc
import concourse.bass as bass
import concourse.tile as tile
import concourse.mybir as mybir
from concourse.bass_utils import run_bass_kernel_spmd

FP32 = mybir.dt.float32
BF16 = mybir.dt.bfloat16
I16 = mybir.dt.int16
AF = mybir.ActivationFunctionType
ALU = mybir.AluOpType

# problem constants
N, IN, HID, H, D = 50000, 128, 256, 4, 64
E, T = 400000, 2
BN_EPS = 1e-5
P = 4                     # cores per quad
NQ = 12500                # real nodes per core
NCP = 12544               # padded (98 * 128)
HS = NCP // 2             # 6272 rows per half of a core's range
SR = P * HS               # 25088 rows per half-slice tensor
NB = NCP // 128           # 98 dst blocks
HB = NB // 2              # 49 blocks per half
GB = 4                    # dst blocks per gather group
ZW = 384                  # packed row: [z 256 | el 4 | er 4 | pad]
ERW = 128                 # replicated er row (bf16 -> 256B)
PADV = 300                # dstv pad marker (outside 0..127)
RGROUPS = [[0, 1, 2, 3], [4, 5, 6, 7]]
import os
STAGES = int(os.environ.get("GNN_STAGES", "99"))


def _bf(x):
    return np.asarray(x, dtype=ml_dtypes.bfloat16)


def _wrap_idx(a):
    """[n] ints (n % 16 == 0) -> [128, n//16] int16 SWDGE wrapped layout
    (token i at [i % 16, i // 16], replicated across the 8 Q7 cores)."""
    w = a.reshape(-1, 16).T.astype(np.int16)
    return np.tile(w, (8, 1))


def _tok_major(a):
    """[n] values (n % 128 == 0) -> [128, n//128] token-major."""
    return a.reshape(-1, 128).T


def _preprocess(inputs):
    feats = np.asarray(inputs["feats"], np.float32)
    edges = [
        (np.asarray(inputs["src0"]), np.asarray(inputs["dst0"])),
        (np.asarray(inputs["src1"]), np.asarray(inputs["dst1"])),
    ]

    # ---- edge buckets per core / dst block / src half-slice ----
    per_core = []
    for q in range(T):
        src, dst = edges[q]
        for r in range(P):
            m = (dst >= r * NQ) & (dst < (r + 1) * NQ)
            g = src[m].astype(np.int64)
            j = (dst[m] - r * NQ).astype(np.int64)
            rs = g // NQ
            is_ = g - rs * NQ
            s = is_ // HS
            row = rs * HS + (is_ - s * HS)      # row in half-slice tensor
            blk = j // 128
            buckets = {}
            for b in range(NB):
                mb_ = blk == b
                for sl in range(2):
                    sel = mb_ & (s == sl)
                    buckets[(b, sl)] = (row[sel], j[sel])
            per_core.append(buckets)

    # shared plan: per (block, slice) padded counts = max over 8 cores
    nbs = np.zeros((NB, 2), np.int64)
    for b in range(NB):
        for sl in range(2):
            mx = max(len(per_core[c][(b, sl)][0]) for c in range(8))
            nbs[b, sl] = ((mx + 127) // 128) * 128

    groups = []
    for g0 in range(0, NB, GB):
        blocks = tuple(range(g0, min(g0 + GB, NB)))
        k0 = int(sum(nbs[b, 0] for b in blocks)) // 128
        k1 = int(sum(nbs[b, 1] for b in blocks)) // 128
        slotmap = []
        for sl in range(2):
            for bi, b in enumerate(blocks):
                slotmap += [(bi, sl)] * (int(nbs[b, sl]) // 128)
        groups.append((blocks, k0, k1, tuple(slotmap)))
    plan_key = tuple(groups)

    ip_cols = []
    for (blocks, k0, k1, _) in groups:
        n0, n1 = k0 * 128, k1 * 128
        ip_cols.append(n0 // 16 + n1 // 16)
    IPW = int(np.sum(ip_cols))
    DVW = int(sum(k0 + k1 for (_, k0, k1, _) in groups))

    fpad = np.zeros((P, NCP, IN), np.float32)
    for rr in range(P):
        fpad[rr, :NQ] = feats[rr * NQ:(rr + 1) * NQ]
    feats_s = np.zeros((2, SR, IN), np.float32)
    for rr in range(P):
        feats_s[0, rr * HS:(rr + 1) * HS] = fpad[rr, 0:HS]
        feats_s[1, rr * HS:(rr + 1) * HS] = fpad[rr, HS:2 * HS]

    in_maps = []
    for c in range(8):
        q, r = c // P, c % P
        buckets = per_core[c]
        ip = np.zeros((128, IPW), np.int16)
        dv = np.zeros((128, DVW), np.float32)
        ipo = 0
        dvo = 0
        for gi, (blocks, k0, k1, _) in enumerate(groups):
            zi = [[], []]
            dvv = []
            for sl in range(2):
                for b in blocks:
                    rows, js = buckets[(b, sl)]
                    n = int(nbs[b, sl])
                    rpad = np.zeros(n, np.int64)
                    dpad = np.full(n, PADV, np.int64)
                    rpad[: len(rows)] = rows
                    dpad[: len(js)] = js - b * 128
                    zi[sl].append(rpad)
                    dvv.append(dpad)
            z0 = (np.concatenate(zi[0]) if zi[0] else np.zeros(0, np.int64))
            z1 = (np.concatenate(zi[1]) if zi[1] else np.zeros(0, np.int64))
            dvs = np.concatenate(dvv)
            for arr in (z0, z1):
                if len(arr):
                    w = _wrap_idx(arr)
                    ip[:, ipo:ipo + w.shape[1]] = w
                    ipo += w.shape[1]
            ns = len(dvs) // 128
            dv[:, dvo:dvo + ns] = _tok_major(dvs.astype(np.float32))
            dvo += ns
        assert ipo == IPW and dvo == DVW, (ipo, IPW, dvo, DVW)

        def gat_wx(Wt, al, ar):
            Wr = Wt.reshape(Wt.shape[0], H, D)
            wal = np.einsum("khd,hd->kh", Wr, al)
            war = np.einsum("khd,hd->kh", Wr, ar)
            wx = np.concatenate([Wt, wal, war], 1)          # [F_in, 264]
            kc = wx.shape[0] // 128
            return _bf(np.ascontiguousarray(
                wx.reshape(kc, 128, 264).transpose(1, 0, 2)))

        def wchunks(Wt):
            kc = Wt.shape[0] // 128
            return _bf(np.ascontiguousarray(
                Wt.reshape(kc, 128, Wt.shape[1]).transpose(1, 0, 2)))

        def fvec(v):
            return np.ascontiguousarray(
                np.asarray(v, np.float32).reshape(2, 128)
                .transpose(1, 0)[:, :, None])

        g = lambda k: np.asarray(inputs[k], np.float32)

        ers0 = _wrap_idx(np.arange(r * HS, (r + 1) * HS, dtype=np.int64))
        ers1 = ers0.copy()

        m = {
            "feats_a0": _bf(feats_s[0]),
            "feats_a1": _bf(feats_s[1]),
            "feats_loc": _bf(fpad[r]),
            "idxpack": ip,
            "dstv": dv.astype(ml_dtypes.bfloat16),
            "ersrc0": ers0,
            "ersrc1": ers1,
            "w0x": gat_wx(g("gat0_W")[q], g("gat0_al")[q], g("gat0_ar")[q]),
            "w1x": gat_wx(g("gat1_W")[q], g("gat1_al")[q], g("gat1_ar")[q]),
            "b0": np.tile(g("gat0_b")[q][None, :], (128, 1)).astype(np.float32),
            "b1": np.tile(g("gat1_b")[q][None, :], (128, 1)).astype(np.float32),
            "g0w1": wchunks(g("gin0_W1")[q]),
            "g0w2": wchunks(g("gin0_W2")[q]),
            "g1w1": wchunks(g("gin1_W1")[q]),
            "g1w2": wchunks(g("gin1_W2")[q]),
            "g0g1": fvec(g("gin0_g1")[q]),
            "g0be1": fvec(g("gin0_be1")[q]),
            "g1g1": fvec(g("gin1_g1")[q]),
            "g1be1": fvec(g("gin1_be1")[q]),
            "g0b2t": np.tile(g("gin0_b2")[q][None, :], (128, 1)).astype(np.float32),
            "g1b2t": np.tile(g("gin1_b2")[q][None, :], (128, 1)).astype(np.float32),
            "eps0": np.full((128, 1), 1.0 + float(g("gin0_eps")[q]), np.float32),
            "eps1": np.full((128, 1), 1.0 + float(g("gin1_eps")[q]), np.float32),
            "identity": _bf(np.eye(128)),
            "identity_f": np.eye(128, dtype=np.float32),
            "iota": _bf(np.tile(np.arange(128, dtype=np.float32)[None, :],
                                (128, 1))),
            "ones_col": _bf(np.ones((128, 1), np.float32)),
            "ones_row": np.ones((1, 128), np.float32),
            "padmask": np.concatenate([
                np.ones((NQ - (NB - 1) * 128, 1), np.float32),
                np.zeros((NCP - NQ, 1), np.float32)]),
        }
        in_maps.append(m)
    return in_maps, (plan_key, IPW, DVW)


def _rows(dram, r0, nt, width):
    return dram[r0 * 128:(r0 + nt) * 128, :].rearrange("(t p) f -> p t f", p=128)


def build_program(plan):
    plan_key, IPW, DVW = plan
    groups = list(plan_key)   # (blocks, k0, k1, slotmap)

    nc = bacc.Bacc("TRN2", target_bir_lowering=False, debug=False,
                   num_devices=8, num_swdge_queues=2)

    dp = nc.declare_dram_parameter
    feats_a = [dp("feats_a0", [SR, IN], BF16, isOutput=False),
               dp("feats_a1", [SR, IN], BF16, isOutput=False)]
    feats_loc_d = dp("feats_loc", [NCP, IN], BF16, isOutput=False)
    ip_d = dp("idxpack", [128, IPW], I16, isOutput=False)
    dv_d = dp("dstv", [128, DVW], BF16, isOutput=False)
    ersrc_d = [dp("ersrc0", [128, HS // 16], I16, isOutput=False),
               dp("ersrc1", [128, HS // 16], I16, isOutput=False)]
    w0x_d = dp("w0x", [128, 1, 264], BF16, isOutput=False)
    w1x_d = dp("w1x", [128, 2, 264], BF16, isOutput=False)
    b0_d = dp("b0", [128, HID], FP32, isOutput=False)
    b1_d = dp("b1", [128, HID], FP32, isOutput=False)
    g0w1_d = dp("g0w1", [128, 3, HID], BF16, isOutput=False)
    g0w2_d = dp("g0w2", [128, 2, HID], BF16, isOutput=False)
    g1w1_d = dp("g1w1", [128, 2, HID], BF16, isOutput=False)
    g1w2_d = dp("g1w2", [128, 2, HID], BF16, isOutput=False)
    vec_d = {nm: dp(nm, [128, 2, 1], FP32, isOutput=False)
             for nm in ("g0g1", "g0be1", "g1g1", "g1be1")}
    b2t_d = {nm: dp(nm, [128, HID], FP32, isOutput=False)
             for nm in ("g0b2t", "g1b2t")}
    eps0_d = dp("eps0", [128, 1], FP32, isOutput=False)
    eps1_d = dp("eps1", [128, 1], FP32, isOutput=False)
    ident_d = dp("identity", [128, 128], BF16, isOutput=False)
    identf_d = dp("identity_f", [128, 128], FP32, isOutput=False)
    iota_d = dp("iota", [128, 128], BF16, isOutput=False)
    onesc_d = dp("ones_col", [128, 1], BF16, isOutput=False)
    onesr_d = dp("ones_row", [1, 128], FP32, isOutput=False)
    padmask_d = dp("padmask", [128, 1], FP32, isOutput=False)

    out_d = dp("out", [NCP, HID], FP32, isOutput=True)

    # DRAM scratch. *_loc tensors are split in row halves so each
    # AllGather half only depends on the blocks that feed it.
    zel0_s = [nc.dram_tensor(f"zel0_s{i}", [SR, ZW], BF16) for i in range(2)]
    zel1_s = [nc.dram_tensor(f"zel1_s{i}", [SR, ZW], BF16) for i in range(2)]
    hcat_s = [nc.dram_tensor(f"hcat_s{i}", [SR, ZW], BF16) for i in range(2)]
    h3_s = [nc.dram_tensor(f"h3_s{i}", [SR, HID], BF16) for i in range(2)]
    zel1_loc = [nc.dram_tensor(f"zel1_loc{i}", [HS, ZW], BF16)
                for i in range(2)]
    hcat_loc = [nc.dram_tensor(f"hcat_loc{i}", [HS, ZW], BF16)
                for i in range(2)]
    h3_loc = [nc.dram_tensor(f"h3_loc{i}", [HS, HID], BF16)
              for i in range(2)]
    er_cmp = [nc.dram_tensor(f"er_cmp{i}", [NCP, 4], BF16)
              for i in range(2)]
    arb_in = [nc.dram_tensor(f"arb_in{i}", [128, 4], FP32) for i in range(2)]
    scl_dram = [nc.dram_tensor(f"scl_dram{i}", [4, 128], FP32)
                for i in range(2)]
    arb_out = [nc.dram_tensor(f"arb_out{i}", [128, 4], FP32) for i in range(2)]

    def loc_rows(halves, b, width):
        """[128, width] AP for dst-block b of a half-split row tensor."""
        half, bb = (0, b) if b < HB else (1, b - HB)
        return halves[half][bb * 128:(bb + 1) * 128, 0:width].rearrange(
            "(t p) f -> p t f", p=128)[:, 0, :]

    ip_off, dv_off = [], []
    o1, o2 = 0, 0
    for (blocks, k0, k1, _) in groups:
        ip_off.append(o1)
        dv_off.append(o2)
        n0, n1 = k0 * 128, k1 * 128
        o1 += n0 // 16 + n1 // 16
        o2 += k0 + k1
    maxslots = max(k0 + k1 for (_, k0, k1, _) in groups)

    with tile.TileContext(nc) as tc:
        cst = tc.alloc_tile_pool(name="cst", bufs=1)

        def ld(dram, shape, dtype):
            t = cst.tile(shape, dtype, tag=dram.name + "_sb")
            nc.sync.dma_start(out=t[:],
                              in_=dram[tuple(slice(None) for _ in shape)])
            return t

        ident = ld(ident_d, [128, 128], BF16)
        identf = ld(identf_d, [128, 128], FP32)
        iota = ld(iota_d, [128, 128], BF16)
        onesc = ld(onesc_d, [128, 1], BF16)
        onesr = ld(onesr_d, [1, 128], FP32)
        padmask = ld(padmask_d, [128, 1], FP32)
        w0x = ld(w0x_d, [128, 1, 264], BF16)
        w1x = ld(w1x_d, [128, 2, 264], BF16)
        b0 = ld(b0_d, [128, HID], FP32)
        b1 = ld(b1_d, [128, HID], FP32)
        g0w1 = ld(g0w1_d, [128, 3, HID], BF16)
        g0w2 = ld(g0w2_d, [128, 2, HID], BF16)
        g1w1 = ld(g1w1_d, [128, 2, HID], BF16)
        g1w2 = ld(g1w2_d, [128, 2, HID], BF16)
        vec = {nm: ld(d, [128, 2, 1], FP32) for nm, d in vec_d.items()}
        b2t = {nm: ld(d, [128, HID], FP32) for nm, d in b2t_d.items()}
        eps0 = ld(eps0_d, [128, 1], FP32)
        eps1 = ld(eps1_d, [128, 1], FP32)

        big = tc.alloc_tile_pool(name="big", bufs=1)
        x1_sb = big.tile([128, NB, HID], BF16, tag="x1_sb")

        # ---------------- GAT0 node: all nodes, no AG ----------------
        def gat0_node():
            with tc.tile_pool(name="n0", bufs=3) as pool, \
                 tc.tile_pool(name="n0p", bufs=2, space="PSUM") as pp:
                for sl in range(2):
                    ntile = SR // 128        # 196
                    for t0 in range(0, ntile, 4):
                        nt = min(4, ntile - t0)
                        ft = pool.tile([128, 4, IN], BF16, tag="ft")
                        nc.sync.dma_start(out=ft[:, 0:nt, :],
                                          in_=_rows(feats_a[sl], t0, nt, IN))
                        zel = pool.tile([128, 4, 264], BF16, tag="zel")
                        for t in range(nt):
                            pt = pp.tile([128, 128], BF16, tag="tp")
                            nc.tensor.transpose(out=pt[:], in_=ft[:, t, :],
                                                identity=ident[:])
                            fT = pool.tile([128, 128], BF16, tag="fT")
                            nc.any.tensor_copy(out=fT[:], in_=pt[:])
                            zp = pp.tile([128, 512], FP32, tag="zp")
                            nc.tensor.matmul(zp[:, 0:264], lhsT=fT[:],
                                             rhs=w0x[:, 0, :],
                                             start=True, stop=True)
                            nc.any.tensor_copy(out=zel[:, t, :],
                                               in_=zp[:, 0:264])
                        nc.sync.dma_start(
                            out=zel0_s[sl][t0 * 128:(t0 + nt) * 128, 0:264]
                            .rearrange("(t p) f -> p t f", p=128),
                            in_=zel[:, 0:nt, :])

        # er_cmp0[i] <- zel0_s[sl][own rows, 260:264]
        def er_fill():
            with tc.tile_pool(name="ef", bufs=2) as pool:
                for sl in range(2):
                    st = pool.tile([128, HS // 16], I16, tag="efst")
                    nc.sync.dma_start(out=st[:], in_=ersrc_d[sl][:, :])
                    zg = pool.tile([128, HS // 128, ZW], BF16, tag="efzg")
                    for p0 in range(0, HS, 896):
                        nc.gpsimd.dma_gather(
                            zg[:, p0 // 128:(p0 + 896) // 128, :],
                            zel0_s[sl][:, :],
                            st[:, p0 // 16:(p0 + 896) // 16], 896, 896, ZW,
                            queue_num=(p0 // 896) % 2)
                    erb = pool.tile([128, HS // 128, 4], BF16, tag="efb")
                    nc.vector.tensor_copy(out=erb[:], in_=zg[:, :, 260:264])
                    nc.sync.dma_start(
                        out=er_cmp[0][sl * HS:(sl + 1) * HS, :].rearrange(
                            "(t p) f -> p t f", p=128),
                        in_=erb[:])

        # hcat_loc cols 256:384 <- feats_loc
        def hcat_prefill():
            with tc.tile_pool(name="hp", bufs=2) as pool:
                for half in range(2):
                    for t0 in range(0, HB, 7):
                        ftl = pool.tile([128, 7, IN], BF16, tag="ftl")
                        nc.sync.dma_start(
                            out=ftl[:],
                            in_=_rows(feats_loc_d, half * HB + t0, 7, IN))
                        nc.sync.dma_start(
                            out=hcat_loc[half][t0 * 128:(t0 + 7) * 128,
                                               256:384]
                            .rearrange("(t p) f -> p t f", p=128),
                            in_=ftl[:])

        MAXTOK = 1024   # SWDGE ring holds 1024 descriptors
        qrr = [0]       # round-robin SWDGE queue cursor

        def gather_split(zg, src_ap, ipt, col0, slot0, ntok, width):
            """dma_gather of ntok tokens in <=MAXTOK pieces (slot-aligned),
            alternating SWDGE queues so the per-queue rings pipeline."""
            done = 0
            while done < ntok:
                take = min(MAXTOK, ntok - done)
                s0 = slot0 + done // 128
                s1 = s0 + (take + 127) // 128
                nc.gpsimd.dma_gather(
                    zg[:, s0:s1, :], src_ap,
                    ipt[:, col0 + done // 16:col0 + (done + take) // 16],
                    take, take, width, queue_num=qrr[0])
                qrr[0] = (qrr[0] + 1) % 2
                done += take

        # ---------------- edge phase ----------------
        def edge_phase(layer, src_s, width, er_src, post, mid_cb=None):
            gat = layer < 2
            rw = 264 if gat else width
            with tc.tile_pool(name=f"e{layer}", bufs=2) as pool, \
                 tc.tile_pool(name=f"e{layer}q", bufs=2) as poolq, \
                 tc.tile_pool(name=f"e{layer}r", bufs=1, space="PSUM") as ppr, \
                 tc.tile_pool(name=f"e{layer}x", bufs=1, space="PSUM") as ppx, \
                 tc.tile_pool(name=f"e{layer}p", bufs=2, space="PSUM") as pp:
                for gi, (blocks, k0, k1, slotmap) in enumerate(groups):
                    ks = k0 + k1
                    n0, n1 = k0 * 128, k1 * 128
                    ipw = n0 // 16 + n1 // 16
                    ipt = poolq.tile([128, ipw], I16, tag="ipt")
                    nc.sync.dma_start(
                        out=ipt[:], in_=ip_d[:, ip_off[gi]:ip_off[gi] + ipw])
                    dvt = poolq.tile([128, maxslots], BF16, tag="dvt")
                    nc.sync.dma_start(
                        out=dvt[:, 0:ks],
                        in_=dv_d[:, dv_off[gi]:dv_off[gi] + ks])
                    zg = pool.tile([128, maxslots, width], BF16, tag="zg")
                    if k0:
                        gather_split(zg, src_s[0][:, :], ipt, 0, 0, n0, width)
                    if k1:
                        gather_split(zg, src_s[1][:, :], ipt, n0 // 16, k0,
                                     n1, width)
                    se = pool.tile([128, maxslots, 128], BF16, tag="se")
                    nc.vector.tensor_tensor(
                        out=se[:, 0:ks, :],
                        in0=dvt[:, 0:ks].unsqueeze(2).broadcast_to(
                            [128, ks, 128]),
                        in1=iota[:].unsqueeze(1).broadcast_to([128, ks, 128]),
                        op=ALU.is_equal)
                    if gat:
                        # er[dst] per token: one-hot SE_T x er_blk on PE
                        erb = poolq.tile([128, GB, 4], BF16, tag="erb")
                        nblk_ = len(blocks)
                        nc.sync.dma_start(
                            out=erb[:, 0:nblk_, :],
                            in_=er_src[blocks[0] * 128:
                                       (blocks[0] + nblk_) * 128, :]
                            .rearrange("(t p) f -> p t f", p=128))
                        seT = pool.tile([128, maxslots, 128], BF16, tag="seT")
                        erp = ppr.tile([128, 512], FP32, tag="erp")
                        for slot, (bi, sl) in enumerate(slotmap):
                            ptT = pp.tile([128, 128], BF16, tag="tp1")
                            nc.tensor.transpose(out=ptT[:],
                                                in_=se[:, slot, :],
                                                identity=ident[:])
                            nc.scalar.copy(out=seT[:, slot, :], in_=ptT[:])
                            nc.tensor.matmul(
                                erp[:, 4 * slot:4 * slot + 4],
                                lhsT=seT[:, slot, :], rhs=erb[:, bi, :],
                                start=(slot == 0), stop=(slot == ks - 1))
                        lg = pool.tile([128, maxslots, H], FP32, tag="lg")
                        nc.vector.tensor_tensor(
                            out=lg[:, 0:ks, :], in0=zg[:, 0:ks, 256:260],
                            in1=erp[:, 0:4 * ks].rearrange(
                                "p (s f) -> p s f", f=4),
                            op=ALU.add)
                        lr = pool.tile([128, maxslots, H], FP32, tag="lr")
                        nc.vector.scalar_tensor_tensor(
                            out=lr[:, 0:ks, :], in0=lg[:, 0:ks, :],
                            scalar=0.2, in1=lg[:, 0:ks, :],
                            op0=ALU.mult, op1=ALU.max)
                        wt = pool.tile([128, maxslots, H], BF16, tag="wt")
                        nc.scalar.activation(out=wt[:, 0:ks, :],
                                             in_=lr[:, 0:ks, :], func=AF.Exp)
                        nc.vector.tensor_tensor(
                            out=zg[:, 0:ks, 0:256].rearrange(
                                "p s (h d) -> p s h d", h=H),
                            in0=zg[:, 0:ks, 0:256].rearrange(
                                "p s (h d) -> p s h d", h=H),
                            in1=wt[:, 0:ks, :].unsqueeze(3).broadcast_to(
                                [128, ks, H, D]),
                            op=ALU.mult)
                        nc.vector.tensor_copy(out=zg[:, 0:ks, 256:260],
                                              in_=wt[:, 0:ks, :])
                    nblk = len(blocks)
                    pbs = [ppr.tile([128, 512], FP32, tag=f"rst{bi}",
                                    name=f"rst{bi}")
                           for bi in range(nblk)]
                    first = [True] * nblk
                    last_slot = {}
                    for slot, (bi, sl) in enumerate(slotmap):
                        last_slot[bi] = slot
                    for slot, (bi, sl) in enumerate(slotmap):
                        nc.tensor.matmul(
                            pbs[bi][:, 0:rw],
                            lhsT=se[:, slot, :], rhs=zg[:, slot, 0:rw],
                            start=first[bi], stop=(slot == last_slot[bi]))
                        first[bi] = False
                    for bi, b in enumerate(blocks):
                        post(b, pbs[bi], pool, pp, ppx)
                    if mid_cb is not None and gi in mid_cb:
                        mid_cb[gi]()

        # ---------------- posts ----------------
        def gat_post(layer):
            bias = b0 if layer == 0 else b1

            def post(b, pb, pool, pp, ppx):
                dmax = pool.tile([128, H], FP32, tag="dmax")
                nc.vector.tensor_scalar_max(dmax[:], pb[:, 256:260], 1e-9)
                rec = pool.tile([128, H], FP32, tag="rec")
                nc.vector.reciprocal(rec[:], dmax[:])
                hb = pool.tile([128, HID], FP32, tag="hb")
                nc.vector.tensor_tensor(
                    out=hb[:].rearrange("p (h d) -> p h d", h=H),
                    in0=pb[:, 0:256].rearrange("p (h d) -> p h d", h=H),
                    in1=rec[:].unsqueeze(2).broadcast_to([128, H, D]),
                    op=ALU.mult)
                hb2 = pool.tile([128, HID], FP32, tag="hb2")
                nc.vector.tensor_tensor(out=hb2[:], in0=hb[:], in1=bias[:],
                                        op=ALU.add)
                hf = pool.tile([128, HID], BF16, tag="hf")
                nc.scalar.activation(out=hf[:], in_=hb2[:], func=AF.Relu)
                if layer == 0:
                    # fused GAT1 node: zel1 = h1 @ w1x
                    hT = pool.tile([128, 2, 128], BF16, tag="hT")
                    for k2 in range(2):
                        pt = pp.tile([128, 128], BF16, tag="tp1")
                        nc.tensor.transpose(
                            out=pt[:], in_=hf[:, k2 * 128:(k2 + 1) * 128],
                            identity=ident[:])
                        nc.any.tensor_copy(out=hT[:, k2, :], in_=pt[:])
                    zp = ppx.tile([128, 512], FP32, tag="z1p")
                    for k2 in range(2):
                        nc.tensor.matmul(zp[:, 0:264], lhsT=hT[:, k2, :],
                                         rhs=w1x[:, k2, :],
                                         start=(k2 == 0), stop=(k2 == 1))
                    z1f = pool.tile([128, 264], BF16, tag="z1f")
                    nc.any.tensor_copy(out=z1f[:], in_=zp[:, 0:264])
                    nc.sync.dma_start(out=loc_rows(zel1_loc, b, 264),
                                      in_=z1f[:])
                    nc.sync.dma_start(
                        out=er_cmp[1][b * 128:(b + 1) * 128, :].rearrange(
                            "(t p) f -> p t f", p=128)[:, 0, :],
                        in_=z1f[:, 260:264])
                else:
                    nc.sync.dma_start(out=loc_rows(hcat_loc, b, 256),
                                      in_=hf[:])
            return post

        def gin_post(layer, stats_pb):
            gidx = layer - 2
            w1 = g0w1 if gidx == 0 else g1w1
            epsv = eps0 if gidx == 0 else eps1
            hc_src = hcat_loc if gidx == 0 else h3_loc
            w_in = 384 if gidx == 0 else 256
            kc = w_in // 128

            def post(b, pb, pool, pp, ppx):
                hcin = pool.tile([128, w_in], BF16, tag="hcin")
                nc.sync.dma_start(out=hcin[:], in_=loc_rows(hc_src, b, w_in))
                xc = pool.tile([128, w_in], BF16, tag="xc")
                nc.vector.scalar_tensor_tensor(
                    out=xc[:], in0=hcin[:], scalar=epsv[:],
                    in1=pb[:, 0:w_in], op0=ALU.mult, op1=ALU.add)
                if b == NB - 1:
                    # zero pad nodes 12500..12543 (partitions 84..127)
                    nc.vector.tensor_tensor(
                        out=xc[:], in0=xc[:],
                        in1=padmask[:].broadcast_to([128, w_in]),
                        op=ALU.mult)
                xT = pool.tile([128, 3, 128], BF16, tag="xT")
                for k2 in range(kc):
                    pt = pp.tile([128, 128], BF16, tag="tp2")
                    nc.tensor.transpose(
                        out=pt[:], in_=xc[:, k2 * 128:(k2 + 1) * 128],
                        identity=ident[:])
                    nc.any.tensor_copy(out=xT[:, k2, :], in_=pt[:])
                xp = ppx.tile([128, 512], FP32, tag="x1p")
                for k2 in range(kc):
                    nc.tensor.matmul(xp[:, 0:HID], lhsT=xT[:, k2, :],
                                     rhs=w1[:, k2, :],
                                     start=(k2 == 0), stop=(k2 == kc - 1))
                x1f = pool.tile([128, HID], BF16, tag="x1f")
                nc.any.tensor_copy(out=x1f[:], in_=xp[:, 0:HID])
                nc.vector.tensor_copy(out=x1_sb[:, b, :], in_=x1f[:])
                sq = pool.tile([128, HID], BF16, tag="sq")
                nc.scalar.activation(out=sq[:], in_=xp[:, 0:HID],
                                     func=AF.Square)
                for col, (srct, chk) in enumerate(
                        ((x1f, 0), (x1f, 1), (sq, 0), (sq, 1))):
                    nc.tensor.matmul(
                        stats_pb[:, col:col + 1],
                        lhsT=srct[:, chk * 128:(chk + 1) * 128], rhs=onesc[:],
                        start=(b == 0 and col == 0),
                        stop=(b == NB - 1 and col == 3))
            return post

        def gin_finish(layer):
            gidx = layer - 2
            w2 = g0w2 if gidx == 0 else g1w2
            pre = "g0" if gidx == 0 else "g1"
            out_f32 = gidx == 1
            with tc.tile_pool(name=f"f{layer}", bufs=3) as pool, \
                 tc.tile_pool(name=f"f{layer}p", bufs=2, space="PSUM") as pp:
                art = pool.tile([128, 4], FP32, tag="art")
                nc.sync.dma_start(out=art[:], in_=arb_out[gidx][:, :])
                mu = pool.tile([128, 2], FP32, tag="mu")
                nc.vector.tensor_scalar_mul(mu[:], art[:, 0:2], 1.0 / N)
                msq = pool.tile([128, 2], FP32, tag="msq")
                nc.vector.tensor_scalar_mul(msq[:], art[:, 2:4], 1.0 / N)
                mu2 = pool.tile([128, 2], FP32, tag="mu2")
                nc.vector.tensor_mul(mu2[:], mu[:], mu[:])
                var = pool.tile([128, 2], FP32, tag="var")
                nc.vector.tensor_sub(var[:], msq[:], mu2[:])
                vare = pool.tile([128, 2], FP32, tag="vare")
                nc.vector.tensor_scalar_add(vare[:], var[:], BN_EPS)
                sd = pool.tile([128, 2], FP32, tag="sd")
                nc.scalar.activation(out=sd[:], in_=vare[:], func=AF.Sqrt)
                rsd = pool.tile([128, 2], FP32, tag="rsd")
                nc.vector.reciprocal(rsd[:], sd[:])
                scl4 = pool.tile([128, 4], FP32, tag="scl4")
                nc.vector.tensor_mul(scl4[:, 0:2], rsd[:],
                                     vec[pre + "g1"][:, :, 0])
                mus = pool.tile([128, 2], FP32, tag="mus")
                nc.vector.tensor_mul(mus[:], mu[:], scl4[:, 0:2])
                nc.vector.tensor_sub(scl4[:, 2:4], vec[pre + "be1"][:, :, 0],
                                     mus[:])
                # broadcast feature-major [128, 4] -> token-major [128, 256]
                ptT = pp.tile([4, 128], FP32, tag="sclTp")
                nc.tensor.transpose(out=ptT[:], in_=scl4[:], identity=identf[:])
                scr = pool.tile([4, 128], FP32, tag="scr")
                nc.any.tensor_copy(out=scr[:], in_=ptT[:])
                # roundtrip rows through DRAM to land each at partition 0
                nc.sync.dma_start(out=scl_dram[gidx][:, :], in_=scr[:])
                sclT = pool.tile([128, HID], FP32, tag="ssclT")
                shfT = pool.tile([128, HID], FP32, tag="sshfT")
                for row, dstt in ((0, sclT), (1, sclT), (2, shfT), (3, shfT)):
                    chk = row % 2
                    srow = pool.tile([1, 128], FP32, tag=f"srow{row}",
                                     name=f"srow{row}")
                    nc.sync.dma_start(out=srow[:],
                                      in_=scl_dram[gidx][row:row + 1, :])
                    bp = pp.tile([128, 128], FP32, tag="bp")
                    nc.tensor.matmul(bp[:], lhsT=onesr[:, :], rhs=srow[:],
                                     start=True, stop=True)
                    nc.any.tensor_copy(out=dstt[:, chk * 128:(chk + 1) * 128],
                                       in_=bp[:])
                # pass B over x1_sb; 7-block tiles stay within row halves
                passb_tiles = list(range(0, NB, 7))
                for t0 in passb_tiles:
                    x1n = pool.tile([128, 7, HID], BF16, tag="x1n")
                    nc.vector.tensor_tensor(
                        out=x1n[:], in0=x1_sb[:, t0:t0 + 7, :],
                        in1=sclT[:].unsqueeze(1).broadcast_to([128, 7, HID]),
                        op=ALU.mult)
                    nc.vector.tensor_tensor(
                        out=x1n[:], in0=x1n[:],
                        in1=shfT[:].unsqueeze(1).broadcast_to([128, 7, HID]),
                        op=ALU.add)
                    nc.scalar.activation(out=x1n[:], in_=x1n[:], func=AF.Relu)
                    ho = pool.tile([128, 7, HID], FP32 if out_f32 else BF16,
                                   tag="ho")
                    for t in range(7):
                        xT = pool.tile([128, 2, 128], BF16, tag="xT2")
                        for k2 in range(2):
                            pt2 = pp.tile([128, 128], BF16, tag="tp3")
                            nc.tensor.transpose(
                                out=pt2[:],
                                in_=x1n[:, t, k2 * 128:(k2 + 1) * 128],
                                identity=ident[:])
                            nc.any.tensor_copy(out=xT[:, k2, :], in_=pt2[:])
                        x2p = pp.tile([128, 512], FP32, tag="x2p")
                        for k2 in range(2):
                            nc.tensor.matmul(x2p[:, 0:HID], lhsT=xT[:, k2, :],
                                             rhs=w2[:, k2, :],
                                             start=(k2 == 0), stop=(k2 == 1))
                        hb3 = pool.tile([128, HID], FP32, tag="hb3")
                        nc.vector.tensor_tensor(out=hb3[:], in0=x2p[:, 0:HID],
                                                in1=b2t[pre + "b2t"][:],
                                                op=ALU.add)
                        nc.scalar.activation(out=ho[:, t, :], in_=hb3[:],
                                             func=AF.Relu)
                    if out_f32:
                        nc.sync.dma_start(out=_rows(out_d, t0, 7, HID),
                                          in_=ho[:])
                    else:
                        half, tt = (0, t0) if t0 < HB else (1, t0 - HB)
                        nc.sync.dma_start(
                            out=_rows(h3_loc[half], tt, 7, HID), in_=ho[:])
                        if t0 + 7 == HB:
                            allgather_half(h3_loc, h3_s, 0)
                        elif t0 + 7 == NB:
                            allgather_half(h3_loc, h3_s, 1)

        def allgather_half(src_halves, dsts, half):
            nc.gpsimd.collective_compute(
                "AllGather", ALU.bypass, replica_groups=RGROUPS,
                ins=[src_halves[half][:, :].opt()],
                outs=[dsts[half][:, :].opt()])

        def allreduce_stats(gidx, stats_pb):
            with tc.tile_pool(name=f"ar{gidx}", bufs=1) as pool:
                arp = pool.tile([128, 4], FP32, tag="arp")
                nc.vector.tensor_copy(out=arp[:], in_=stats_pb[:, 0:4])
                nc.sync.dma_start(out=arb_in[gidx][:, :], in_=arp[:])
            nc.gpsimd.collective_compute(
                "AllReduce", ALU.add, replica_groups=RGROUPS,
                ins=[arb_in[gidx][:, :].opt()],
                outs=[arb_out[gidx][:, :].opt()])

        # ---------------- schedule ----------------
        # half 0 = dst blocks 0..48; group 12 = blocks 48..51 completes it
        gat0_node()
        hcat_prefill()
        er_fill()
        if STAGES >= 2:
            edge_phase(0, zel0_s, ZW, er_cmp[0], gat_post(0),
                       mid_cb={12: lambda: allgather_half(zel1_loc, zel1_s, 0),
                               24: lambda: allgather_half(zel1_loc, zel1_s, 1)})
        if STAGES >= 3:
            edge_phase(1, zel1_s, ZW, er_cmp[1], gat_post(1),
                       mid_cb={12: lambda: allgather_half(hcat_loc, hcat_s, 0),
                               24: lambda: allgather_half(hcat_loc, hcat_s, 1)})
        if STAGES >= 4:
            with tc.tile_pool(name="sp0", bufs=1, space="PSUM") as sp:
                stats0 = sp.tile([128, 512], FP32, tag="stats0")
                edge_phase(2, hcat_s, ZW, None, gin_post(2, stats0))
                allreduce_stats(0, stats0)
            gin_finish(2)
        if STAGES >= 5:
            with tc.tile_pool(name="sp1", bufs=1, space="PSUM") as sp:
                stats1 = sp.tile([128, 512], FP32, tag="stats1")
                edge_phase(3, h3_s, HID, None, gin_post(3, stats1))
                allreduce_stats(1, stats1)
            gin_finish(3)

        big.release()
        cst.release()

    nc.compile()
    return nc


_CACHE = {}


def kernel(**inputs):
    in_maps, plan = _preprocess(inputs)
    nc = _CACHE.get(plan[0])
    if nc is None:
        nc = build_program(plan)
        _CACHE[plan[0]] = nc
    res = run_bass_kernel_spmd(nc, in_maps, core_ids=list(range(8)))
    out = np.zeros((N, T * HID), np.float32)
    for c in range(8):
        q, r = c // P, c % P
        out[r * NQ:(r + 1) * NQ, q * HID:(q + 1) * HID] = \
            np.asarray(res.results[c]["out"], np.float32)[:NQ]
    return out
